# revision 3
# baseline (speedup 1.0000x reference)
"""Two-layer GAT (DGL-style) on 8 TRN2 NeuronCores via Bass/Tile.

Strategy (edge-parallel / dst-sharded hybrid):
- Host: degree-stratified round-robin assignment of dst nodes to the 8 cores:
  for every degree class, each core receives ceil(n_g/8) dsts (padded with
  dummy slots), so all cores share ONE static program structure and perfectly
  balanced edge counts. Each core owns its dsts' output rows completely (no
  cross-core reductions).
- Per-core node permutation: table rows = [own dst slots | all other nodes |
  zeros pad | dummy]. Feature tables ([h bf16 256 | el bf16 8], 768B row
  stride) are built on-device by a replicated matmul; per-edge rows are
  fetched with the InstDMAGatherAnt custom gather (int16 indices, two
  half-table views), dst-side er via a second small gather (256B rows).
- Segment softmax without max-subtraction (logits are O(1), exp is safe):
  out[d] = sum_e exp(lrelu(el+er)) * h[src] / sum_e exp(lrelu(el+er)),
  accumulated per super-chunk (<=128 dst slots, <=2048 edges) with staircase
  0/1 S-matrices built by is_equal(iota, dst_local) and PE matmuls into PSUM.
- Between layers: elu+head-mean pass writes h1^T, AllGather across the 8
  cores, then the layer-2 table is built from the gathered activations.
- Host finalizes: divide by the gathered exp-sums, add bias, inverse-permute.
"""

import numpy as np
import ml_dtypes

bf16 = ml_dtypes.bfloat16
LAST_RESULTS = None
P = 128
SC_E = 2048          # max edges per super-chunk
SC_SLOTS = 128       # max dst slots per super-chunk
NC_CORES = 8
EPS = 1e-30


# ----------------------------------------------------------------- planning

class Plan:
    pass


def _plan(dst, n_nodes):
    pl = Plan()
    deg = np.bincount(dst, minlength=n_nodes)
    order = np.argsort(-deg, kind="stable")
    degs = deg[order]
    percore = [[] for _ in range(NC_CORES)]
    degseq = []
    i = 0
    while i < n_nodes:
        g = int(degs[i])
        j = i
        while j < n_nodes and degs[j] == g:
            j += 1
        nodes = order[i:j]
        m = -(-(j - i) // NC_CORES)
        padded = np.full(m * NC_CORES, -1, np.int64)
        padded[: j - i] = nodes
        for c in range(NC_CORES):
            percore[c].extend(padded[c * m:(c + 1) * m].tolist())
        degseq.extend([g] * m)
        i = j
    # pad D to a multiple of 128 with degree-0 dummy slots
    D = len(degseq)
    pad = (-D) % P
    degseq.extend([0] * pad)
    for c in range(NC_CORES):
        percore[c].extend([-1] * pad)
    D += pad
    pl.D, pl.degseq, pl.percore, pl.deg = D, degseq, percore, deg

    # super-chunk packing over the shared degree sequence
    scs = []
    i = 0
    while i < D:
        n, e = 0, 0
        while i + n < D and n < SC_SLOTS:
            g = degseq[i + n]
            if e + g > SC_E and n > 0:
                break
            e += g
            n += 1
        scs.append({"start": i, "nslots": n, "edges": e})
        i += n
    pl.scs = scs

    # table-1 row layout per core: [slots 0..D) | rest nodes | pad | dummy]
    pl.NROWS1 = ((D + n_nodes + 1 + P - 1) // P) * P
    pl.HALF1 = min(25600, ((pl.NROWS1 // 2) // P) * P)
    assert pl.NROWS1 - pl.HALF1 <= 32768 and pl.HALF1 <= 32768
    pl.NROWS2 = ((NC_CORES * D + 1 + P - 1) // P) * P
    pl.HALF2 = min(25600, ((pl.NROWS2 // 2) // P) * P)
    assert pl.NROWS2 - pl.HALF2 <= 32768 and pl.HALF2 <= 32768
    pl.n_nodes = n_nodes
    return pl


def _edge_lists(pl, src, dst):
    """Per dst node: list of srcs (sorted stable)."""
    order = np.argsort(dst, kind="stable")
    ds, ss = dst[order], src[order]
    bounds = np.searchsorted(ds, np.arange(pl.n_nodes + 1))
    return ss, bounds


def _node2row(pl, c, layer):
    """Map global node id -> table row, for core c and the given layer."""
    n2r = np.full(pl.n_nodes + 1, -1, np.int64)
    if layer == 1:
        own = set()
        for i, d in enumerate(pl.percore[c]):
            if d >= 0:
                n2r[d] = i
                own.add(d)
        k = pl.D
        for nd in range(pl.n_nodes):
            if nd not in own:
                n2r[nd] = k
                k += 1
        dummy = pl.NROWS1 - 1
    else:
        for cc in range(NC_CORES):
            for i, d in enumerate(pl.percore[cc]):
                if d >= 0:
                    n2r[d] = cc * pl.D + i
        dummy = pl.NROWS2 - 1
    return n2r, dummy


def _streams_for_layer(pl, src, dst, layer):
    """Build per-core edge streams; returns per-sc capacities (shared) and
    per-core arrays. Stream edge order: all A-half edges, then B-half."""
    ss, bounds = _edge_lists(pl, src, dst)
    HALF = pl.HALF1 if layer == 1 else pl.HALF2
    percore_sc = []  # [core][sc] -> dict(A=[(row, slot)], B=[...])
    for c in range(NC_CORES):
        n2r, dummy = _node2row(pl, c, layer)
        sc_items = []
        for sc in pl.scs:
            A, B = [], []
            for j in range(sc["nslots"]):
                d = pl.percore[c][sc["start"] + j]
                if d < 0:
                    continue
                for s in ss[bounds[d]:bounds[d + 1]]:
                    r = n2r[s]
                    (A if r < HALF else B).append((int(r) if r < HALF else int(r) - HALF, j))
            sc_items.append({"A": A, "B": B})
        percore_sc.append(sc_items)
    CA = [max(1, -(-max(len(percore_sc[c][s]["A"]) for c in range(NC_CORES)) // P)) * P
          for s in range(len(pl.scs))]
    CB = [max(1, -(-max(len(percore_sc[c][s]["B"]) for c in range(NC_CORES)) // P)) * P
          for s in range(len(pl.scs))]
    return percore_sc, CA, CB


def _wrap16(a):
    """Logical index array (len multiple of 16) -> [128, n/16] int16 tile."""
    w = np.asarray(a, np.int16).reshape(-1, 16).T
    return np.tile(w, (8, 1)).copy()


def _pack_core_layer(pl, percore_sc_c, CA, CB):
    """Per-core per-layer input arrays: idxA, idxB, idxE, dl, cnt."""
    iA, iB, iE, DL, CNT = [], [], [], [], []
    for s, sc in enumerate(pl.scs):
        A, B = percore_sc_c[s]["A"], percore_sc_c[s]["B"]
        nA, nB = len(A), len(B)
        ca, cb = CA[s], CB[s]
        T = ca + cb
        ia = np.full(ca, -1, np.int16)
        ia[:nA] = [r for r, _ in A]
        ib = np.full(cb, -1, np.int16)
        ib[:nB] = [r for r, _ in B]
        # dma_gather with a zero valid count is risky; gather one dummy row
        # into a dead (dl=-1) position instead.
        if nA == 0:
            ia[0] = 0
            nA = 1
        if nB == 0:
            ib[0] = 0
            nB = 1
        dl = np.full(T, -1.0, np.float32)
        ie = np.zeros(T, np.int16)
        for pos, (r, j) in enumerate(A):
            dl[pos] = j
            ie[pos] = sc["start"] + j
        for pos, (r, j) in enumerate(B):
            dl[ca + pos] = j
            ie[ca + pos] = sc["start"] + j
        iA.append(_wrap16(ia))
        iB.append(_wrap16(ib))
        iE.append(_wrap16(ie))
        # dl laid out by tile position: [128, T/128] where [p, c] = pos c*128+p
        DL.append(dl.reshape(T // P, P).T.astype(bf16).copy())
        CNT.extend([nA, nB])
    return (np.concatenate(iA, 1), np.concatenate(iB, 1), np.concatenate(iE, 1),
            np.concatenate(DL, 1), np.asarray(CNT, np.int32).reshape(1, -1))


def _host_weights(W1, al1, ar1, W2, al2, ar2, F1, H, F2):
    """rhs1 [IN_F, 272] bf16, rhs2 [F1, 272] bf16, corr2 [272] f32."""
    W1_ = W1.astype(np.float64)
    W1r = W1_.reshape(W1.shape[0], H, F1)
    C1el_ = np.einsum("khf,hf->kh", W1r, al1.astype(np.float64))
    C1er_ = np.einsum("khf,hf->kh", W1r, ar1.astype(np.float64))
    rhs1 = np.concatenate([W1_, C1el_, C1er_], 1)  # [IN_F, 272]
    M2 = np.concatenate(
        [W2.astype(np.float64),
         np.einsum("khf,hf->kh", W2.astype(np.float64).reshape(F1, H, F2), al2.astype(np.float64)),
         np.einsum("khf,hf->kh", W2.astype(np.float64).reshape(F1, H, F2), ar2.astype(np.float64))], 1)
    # pass-2 produces h1_true = mean_h(elu) in f32 directly, so no scale/corr
    rhs2 = M2
    corr2 = np.zeros(M2.shape[1])
    return rhs1.astype(bf16), rhs2.astype(bf16), corr2.astype(np.float32)


def _build_xT(pl, c, x, IN_F):
    n2r, _ = _node2row(pl, c, 1)
    xT = np.zeros((IN_F, pl.NROWS1), bf16)
    rows = n2r[:pl.n_nodes]
    xT[:, rows] = x.T.astype(bf16)
    return xT


# ----------------------------------------------------------- numpy mirror

def _sim_core_layer(pl, table, ert, arrs, CA, CB):
    """Mirror of the device edge phase for one core/layer.
    table: [NROWS, 264] float32-valued (bf16-rounded h|el); ert: [D, 8] f32.
    arrs: (iA, iB, iE, dl, cnt) as packed. Returns raw [D, 264] f32."""
    iA, iB, iE, DL, CNT = arrs
    HALFROWS = table.shape[0]
    raw = np.zeros((pl.D, 264), np.float32)
    colA = colB = colE = colD = 0
    for s, sc in enumerate(pl.scs):
        ca, cb = CA[s], CB[s]
        T = ca + cb
        nA, nB = CNT[0, 2 * s], CNT[0, 2 * s + 1]
        ia = iA[:16, colA:colA + ca // 16].T.reshape(-1)
        ib = iB[:16, colB:colB + cb // 16].T.reshape(-1)
        ie = iE[:16, colE:colE + T // 16].T.reshape(-1)
        dl = DL[:, colD:colD + T // P].T.reshape(-1).astype(np.float32)  # pos-major
        # careful: DL[p, c] = pos c*128+p -> transpose gives pos order
        colA += ca // 16
        colB += cb // 16
        colE += T // 16
        colD += T // P
        G = np.zeros((T, 264), np.float32)
        half = pl._half_cur
        for i in range(nA):
            G[i] = table[ia[i]]
        for i in range(nB):
            G[ca + i] = table[half + int(ib[i])]
        er = ert[np.maximum(ie, 0)]  # [T, 8]
        z = G[:, 256:264] + er
        z = np.where(z >= 0, z, 0.2 * z)
        ex = np.exp(z).astype(bf16).astype(np.float32)
        S = (dl[:, None] == np.arange(P)[None, :])
        V = np.concatenate([
            (ex[:, :, None] * G[:, :256].reshape(T, 8, 32)).reshape(T, 256)
            .astype(bf16).astype(np.float32), ex.astype(bf16).astype(np.float32)], 1)
        acc = S.T.astype(np.float32) @ V  # [128, 264]
        m = sc["nslots"]
        raw[sc["start"]:sc["start"] + m] += acc[:m]
    return raw


def _simulate(pl, inputs, dcore):
    """Full numpy mirror across cores (bf16-faithful where it matters)."""
    x, src, dst = inputs["x"], inputs["src"], inputs["dst"]
    W1, al1, ar1, b1 = inputs["W1"], inputs["al1"], inputs["ar1"], inputs["b1"]
    W2, al2, ar2, b2 = inputs["W2"], inputs["al2"], inputs["ar2"], inputs["b2"]
    H = al1.shape[0]
    F1 = al1.shape[1]
    F2 = al2.shape[1]
    IN_F = x.shape[1]
    rhs1, rhs2, corr2 = _host_weights(W1, al1, ar1, W2, al2, ar2, F1, H, F2)
    sc1, CA1, CB1 = dcore["sc1"], dcore["CA1"], dcore["CB1"]
    sc2, CA2, CB2 = dcore["sc2"], dcore["CA2"], dcore["CB2"]

    h1T_all = np.zeros((NC_CORES, pl.D, F1), np.float32)
    for c in range(NC_CORES):
        xT = _build_xT(pl, c, x, IN_F)
        ps = xT.T.astype(np.float32) @ rhs1.astype(np.float32)  # [NROWS1, 272]
        table = np.zeros((pl.NROWS1, 264), np.float32)
        table[:, :264] = ps[:, :264].astype(bf16).astype(np.float32)
        ert = ps[:pl.D, 264:272].astype(np.float32)
        pl._half_cur = pl.HALF1
        arrs = dcore["l1"][c]
        raw = _sim_core_layer(pl, table, ert, arrs, CA1, CB1)
        rec = 1.0 / (raw[:, 256:] + EPS)
        o1 = raw[:, :256].reshape(pl.D, H, F1) * rec[:, :, None] + b1.reshape(1, H, F1)
        o1 = o1.astype(bf16).astype(np.float32)
        u = np.where(o1 > 0, o1, np.exp(np.minimum(o1, 0)) - 1) + 1.0
        h1T_all[c] = u.sum(1) / H - 1.0  # = mean_h(elu), in f32
    # allgather order [core, slot]
    h2ps = np.concatenate([h1T_all[c] for c in range(NC_CORES)], 0)  # [8D, F1]
    h2ps = np.concatenate([h2ps, np.zeros((pl.NROWS2 - NC_CORES * pl.D, F1), np.float32)], 0)
    ps2 = h2ps.astype(bf16).astype(np.float32) @ rhs2.astype(np.float32) + corr2[None, :]
    table2 = ps2[:, :264].astype(bf16).astype(np.float32)
    table2[NC_CORES * pl.D:] = 0.0  # dummy/pad rows built as zeros on device
    out = np.zeros((pl.n_nodes, H, F2), np.float32)
    for c in range(NC_CORES):
        er2 = (h1T_all[c].astype(bf16).astype(np.float32) @ rhs2[:, 264:272].astype(np.float32)
               + corr2[None, 264:272]).astype(np.float32)  # [D, 8]
        pl._half_cur = pl.HALF2
        raw = _sim_core_layer(pl, table2, er2, dcore["l2"][c], CA2, CB2)
        o2 = raw[:, :256].reshape(pl.D, H, F2) / np.maximum(raw[:, 256:], EPS)[:, :, None] \
            + b2.reshape(1, H, F2)
        for i, d in enumerate(pl.percore[c]):
            if d >= 0:
                out[d] = o2[i]
    return out


# ----------------------------------------------------------- bass program

def _build_bass(pl, n_sc_arrs, IN_F, H, F1, F2):
    import concourse.bass as bass
    import concourse.bacc as bacc
    import concourse.mybir as mybir
    import concourse.tile as tile
    from concourse.masks import make_identity

    CA1, CB1, CA2, CB2 = (n_sc_arrs[k] for k in ("CA1", "CB1", "CA2", "CB2"))
    f32, i16, i32, bfl = (mybir.dt.float32, mybir.dt.int16, mybir.dt.int32,
                          mybir.dt.bfloat16)
    FEAT = H * F1          # 256
    FCOL = FEAT + 8        # 264 used cols
    ROWW = 384             # table row stride (768 B)
    NT1 = pl.NROWS1 // P
    NT2 = pl.NROWS2 // P
    ND = pl.D // P
    n_sc = len(pl.scs)
    sumT1 = sum(CA1[s] + CB1[s] for s in range(n_sc))
    sumT2 = sum(CA2[s] + CB2[s] for s in range(n_sc))

    nc = bacc.Bacc("TRN2", target_bir_lowering=False)
    # inputs
    t_xT = nc.dram_tensor("xT", [IN_F, pl.NROWS1], bfl, kind="ExternalInput")
    t_rhs1 = nc.dram_tensor("rhs1", [IN_F, 272], bfl, kind="ExternalInput")
    t_rhs2 = nc.dram_tensor("rhs2", [F1, 272], bfl, kind="ExternalInput")
    t_corr2 = nc.dram_tensor("corr2", [P, 272], f32, kind="ExternalInput")
    t_b1 = nc.dram_tensor("b1rep", [P, FEAT], f32, kind="ExternalInput")
    t_iota = nc.dram_tensor("iota", [P, P], bfl, kind="ExternalInput")
    t_iA1 = nc.dram_tensor("iA1", [P, sum(CA1) // 16], i16, kind="ExternalInput")
    t_iB1 = nc.dram_tensor("iB1", [P, sum(CB1) // 16], i16, kind="ExternalInput")
    t_iE1 = nc.dram_tensor("iE1", [P, sumT1 // 16], i16, kind="ExternalInput")
    t_dl1 = nc.dram_tensor("dl1", [P, sumT1 // P], bfl, kind="ExternalInput")
    t_cn1 = nc.dram_tensor("cn1", [1, 2 * n_sc], i32, kind="ExternalInput")
    t_iA2 = nc.dram_tensor("iA2", [P, sum(CA2) // 16], i16, kind="ExternalInput")
    t_iB2 = nc.dram_tensor("iB2", [P, sum(CB2) // 16], i16, kind="ExternalInput")
    t_iE2 = nc.dram_tensor("iE2", [P, sumT2 // 16], i16, kind="ExternalInput")
    t_dl2 = nc.dram_tensor("dl2", [P, sumT2 // P], bfl, kind="ExternalInput")
    t_cn2 = nc.dram_tensor("cn2", [1, 2 * n_sc], i32, kind="ExternalInput")
    # outputs
    t_out = nc.dram_tensor("raw2", [pl.D, FCOL], f32, kind="ExternalOutput")
    # internal DRAM
    t_tab1 = nc.dram_tensor("tab1", [pl.NROWS1, ROWW], bfl)
    t_tab2 = nc.dram_tensor("tab2", [pl.NROWS2, ROWW], bfl)
    t_er1 = nc.dram_tensor("er1t", [pl.D, 64], f32)
    t_er2 = nc.dram_tensor("er2t", [pl.D, 64], f32)
    t_raw1 = nc.dram_tensor("raw1", [pl.D, FCOL], f32)
    t_agin = nc.dram_tensor("agin", [F1, pl.D], f32)
    t_ag = nc.dram_tensor("ag", [NC_CORES, F1, pl.D], f32, addr_space="Shared")

    with tile.TileContext(nc) as tc:
        with (tc.tile_pool(name="const", bufs=1) as cpool,
              tc.tile_pool(name="tb", bufs=3) as tb,
              tc.tile_pool(name="edge", bufs=2) as ep,
              tc.tile_pool(name="psum", bufs=2, space="PSUM") as pp,
              tc.tile_pool(name="psum1", bufs=2, space="PSUM") as pp1):
            # ---- constants in SBUF
            rhs1_t = cpool.tile([IN_F, 272], bfl)
            nc.sync.dma_start(out=rhs1_t[:], in_=t_rhs1[:])
            rhs2_t = cpool.tile([F1, 272], bfl)
            nc.sync.dma_start(out=rhs2_t[:], in_=t_rhs2[:])
            corr2_t = cpool.tile([P, 272], f32)
            nc.sync.dma_start(out=corr2_t[:], in_=t_corr2[:])
            b1_t = cpool.tile([P, FEAT], f32)
            nc.sync.dma_start(out=b1_t[:], in_=t_b1[:])
            iota_t = cpool.tile([P, P], bfl)
            nc.sync.dma_start(out=iota_t[:], in_=t_iota[:])
            ident_t = cpool.tile([P, P], f32)
            make_identity(nc, ident_t[:])
            cn1_t = cpool.tile([1, 2 * n_sc], i32)
            nc.sync.dma_start(out=cn1_t[:], in_=t_cn1[:])
            cn2_t = cpool.tile([1, 2 * n_sc], i32)
            nc.sync.dma_start(out=cn2_t[:], in_=t_cn2[:])

            # ---- table 1 build: tile t covers rows t*128..t*128+127
            for t in range(NT1):
                xt = tb.tile([IN_F, P], bfl, tag="xt")
                nc.sync.dma_start(out=xt[:], in_=t_xT[:, t * P:(t + 1) * P])
                ps = pp.tile([P, 272], f32, tag="tps")
                nc.tensor.matmul(ps[:], lhsT=xt[:], rhs=rhs1_t[:],
                                 start=True, stop=True)
                tt = tb.tile([P, FCOL], bfl, tag="tt")
                nc.vector.tensor_copy(out=tt[:], in_=ps[:, :FCOL])
                nc.sync.dma_start(
                    out=t_tab1[t * P:(t + 1) * P, :FCOL], in_=tt[:])
                if t < ND:
                    ert = tb.tile([P, 8], f32, tag="ert")
                    nc.vector.tensor_copy(out=ert[:], in_=ps[:, FCOL:272])
                    nc.sync.dma_start(
                        out=t_er1[t * P:(t + 1) * P, 0:8], in_=ert[:])

            # ---- layer-1 edge phase
            _edge_phase(nc, tc, ep, pp1, pl, CA1, CB1, t_tab1, t_er1, t_raw1,
                        t_iA1, t_iB1, t_iE1, t_dl1, cn1_t, iota_t,
                        pl.HALF1, FEAT, FCOL, ROWW, mybir, bass, layer=1)

            # ---- pass 2: raw1 -> h1' -> agin (transposed)
            for t in range(ND):
                rt = tb.tile([P, FCOL], f32, tag="rt")
                nc.sync.dma_start(out=rt[:], in_=t_raw1[t * P:(t + 1) * P, :])
                se = tb.tile([P, 8], f32, tag="se")
                nc.vector.tensor_scalar_add(out=se[:], in0=rt[:, FEAT:FCOL],
                                            scalar1=EPS)
                rec = tb.tile([P, 8], f32, tag="rec")
                nc.vector.reciprocal(out=rec[:], in_=se[:])
                o1 = tb.tile([P, FEAT], bfl, tag="o1")
                nc.vector.tensor_tensor(
                    out=o1[:].rearrange("p (h f) -> p h f", f=F1),
                    in0=rt[:, :FEAT].rearrange("p (h f) -> p h f", f=F1),
                    in1=rec[:].unsqueeze(2).to_broadcast([P, H, F1]),
                    op=mybir.AluOpType.mult)
                nc.vector.tensor_tensor(out=o1[:], in0=o1[:], in1=b1_t[:],
                                        op=mybir.AluOpType.add)
                r_ = tb.tile([P, FEAT], bfl, tag="r_")
                nc.scalar.activation(out=r_[:], in_=o1[:],
                                     func=mybir.ActivationFunctionType.Relu,
                                     scale=-1.0)
                tex = tb.tile([P, FEAT], bfl, tag="tex")
                nc.scalar.activation(out=tex[:], in_=r_[:],
                                     func=mybir.ActivationFunctionType.Exp,
                                     scale=-1.0)
                nc.vector.tensor_tensor(out=o1[:], in0=o1[:], in1=r_[:],
                                        op=mybir.AluOpType.add)
                nc.vector.tensor_tensor(out=o1[:], in0=o1[:], in1=tex[:],
                                        op=mybir.AluOpType.add)
                hs = tb.tile([P, F1], f32, tag="hs")
                nc.vector.tensor_reduce(
                    out=hs[:],
                    in_=o1[:].rearrange("p (h f) -> p f h", f=F1),
                    axis=mybir.AxisListType.X, op=mybir.AluOpType.add)
                # h1_true = hs/H - 1 in f32 (cast to bf16 only at consumption;
                # doing -1 here avoids catastrophic cancellation in bf16)
                nc.vector.tensor_scalar(
                    out=hs[:], in0=hs[:], scalar1=1.0 / H, scalar2=-1.0,
                    op0=mybir.AluOpType.mult, op1=mybir.AluOpType.add)
                pst = pp.tile([P, P], f32, tag="pst")
                nc.tensor.transpose(out=pst[:F1, :], in_=hs[:],
                                    identity=ident_t[:])
                ht = tb.tile([F1, P], f32, tag="ht")
                nc.vector.tensor_copy(out=ht[:], in_=pst[:F1, :])
                nc.sync.dma_start(out=t_agin[:, t * P:(t + 1) * P], in_=ht[:])

            # ---- allgather
            nc.gpsimd.collective_compute(
                "AllGather", mybir.AluOpType.bypass,
                replica_groups=[list(range(NC_CORES))],
                ins=[t_agin[:]], outs=[t_ag[:]])

            # ---- er2 table from local h1'
            for t in range(ND):
                hbt = tb.tile([F1, P], bfl, tag="hbt")
                nc.gpsimd.dma_start(out=hbt[:], in_=t_agin[:, t * P:(t + 1) * P])
                ps8 = pp.tile([P, 8], f32, tag="ps8")
                nc.tensor.matmul(ps8[:], lhsT=hbt[:], rhs=rhs2_t[:, FCOL:272],
                                 start=True, stop=True)
                er2 = tb.tile([P, 8], f32, tag="er2")
                nc.vector.tensor_tensor(out=er2[:], in0=ps8[:],
                                        in1=corr2_t[:, FCOL:272],
                                        op=mybir.AluOpType.add)
                nc.sync.dma_start(out=t_er2[t * P:(t + 1) * P, 0:8], in_=er2[:])

            # ---- table 2 build
            for t in range(NT2):
                r0 = t * P
                if r0 >= NC_CORES * pl.D:
                    zt = tb.tile([P, FCOL], bfl, tag="tt")
                    nc.gpsimd.memset(zt[:], 0)
                    nc.sync.dma_start(out=t_tab2[r0:r0 + P, :FCOL], in_=zt[:])
                    continue
                cb_ = r0 // pl.D
                i0 = r0 % pl.D
                hbt = tb.tile([F1, P], bfl, tag="hbt")
                nc.gpsimd.dma_start(
                    out=hbt[:], in_=t_ag[cb_, :, i0:i0 + P])
                ps = pp.tile([P, 272], f32, tag="tps")
                nc.tensor.matmul(ps[:], lhsT=hbt[:], rhs=rhs2_t[:],
                                 start=True, stop=True)
                tt = tb.tile([P, FCOL], bfl, tag="tt")
                nc.vector.tensor_tensor(out=tt[:], in0=ps[:, :FCOL],
                                        in1=corr2_t[:, :FCOL],
                                        op=mybir.AluOpType.add)
                nc.sync.dma_start(out=t_tab2[r0:r0 + P, :FCOL], in_=tt[:])

            # ---- layer-2 edge phase
            _edge_phase(nc, tc, ep, pp1, pl, CA2, CB2, t_tab2, t_er2, t_out,
                        t_iA2, t_iB2, t_iE2, t_dl2, cn2_t, iota_t,
                        pl.HALF2, FEAT, FCOL, ROWW, mybir, bass, layer=2)

    nc.compile()
    return nc


def _edge_phase(nc, tc, ep, pp1, pl, CA, CB, t_tab, t_ert, t_raw,
                t_iA, t_iB, t_iE, t_dl, cn_t, iota_t, HALF,
                FEAT, FCOL, ROWW, mybir, bass, layer):
    f32, i16, bfl = mybir.dt.float32, mybir.dt.int16, mybir.dt.bfloat16
    n_sc = len(pl.scs)
    maxT = max(CA[s] + CB[s] for s in range(n_sc))
    rA = nc.alloc_register(mybir.EngineType.Pool, f"nA{layer}")
    rB = nc.alloc_register(mybir.EngineType.Pool, f"nB{layer}")
    offA = offB = offE = offD = 0
    bufs_seen = 0
    for s, sc in enumerate(pl.scs):
        ca, cb = CA[s], CB[s]
        T = ca + cb
        nch = T // P
        ncha = ca // P
        oA, oB, oE, oD = offA, offB, offE, offD
        offA += ca // 16
        offB += cb // 16
        offE += T // 16
        offD += nch
        m = sc["nslots"]
        if sc["edges"] == 0:
            zro = ep.tile([P, FCOL], f32, tag="ro")
            nc.vector.memset(zro[:], 0)
            nc.sync.dma_start(out=t_raw[sc["start"]:sc["start"] + m, :],
                              in_=zro[:m, :])
            continue
        ia = ep.tile([P, ca // 16], i16, tag="ia")
        nc.sync.dma_start(out=ia[:], in_=t_iA[:, oA:oA + ca // 16])
        ib = ep.tile([P, cb // 16], i16, tag="ib")
        nc.sync.dma_start(out=ib[:], in_=t_iB[:, oB:oB + cb // 16])
        ie = ep.tile([P, T // 16], i16, tag="ie")
        nc.sync.dma_start(out=ie[:], in_=t_iE[:, oE:oE + T // 16])
        dl = ep.tile([P, nch], bfl, tag="dl")
        nc.sync.dma_start(out=dl[:], in_=t_dl[:, oD:oD + nch])

        G = ep.tile([P, (maxT // P) * ROWW], bfl, tag="G")
        if bufs_seen < 2:
            nc.gpsimd.memset(G[:], 0)
        ER = ep.tile([P, (maxT // P) * 64], f32, tag="ER")
        if bufs_seen < 2:
            nc.gpsimd.memset(ER[:], 0)
            bufs_seen += 1
        nc.gpsimd.reg_load(rA, cn_t[0:1, 2 * s:2 * s + 1])
        nc.gpsimd.dma_gather(
            G[:, :ncha * ROWW].rearrange("p (c e) -> p c e", e=ROWW),
            t_tab[:HALF, :], ia[:], num_idxs=ca, num_idxs_reg=rA,
            elem_size=ROWW, single_packet=False)
        nc.gpsimd.reg_load(rB, cn_t[0:1, 2 * s + 1:2 * s + 2])
        nc.gpsimd.dma_gather(
            G[:, ncha * ROWW:nch * ROWW].rearrange("p (c e) -> p c e", e=ROWW),
            t_tab[HALF:, :], ib[:], num_idxs=cb, num_idxs_reg=rB,
            elem_size=ROWW, single_packet=False)
        nc.gpsimd.dma_gather(
            ER[:, :nch * 64].rearrange("p (c e) -> p c e", e=64),
            t_ert[:, :], ie[:], num_idxs=T, num_idxs_reg=T,
            elem_size=64, single_packet=False)

        Gv = G[:, :nch * ROWW].rearrange("p (c e) -> p c e", e=ROWW)
        ERv = ER[:, :nch * 64].rearrange("p (c e) -> p c e", e=64)
        z = ep.tile([P, nch * 8], f32, tag="z")
        nc.vector.tensor_tensor(
            out=z[:].rearrange("p (c h) -> p c h", h=8),
            in0=Gv[:, :, FEAT:FCOL], in1=ERv[:, :, 0:8],
            op=mybir.AluOpType.add)
        lr = ep.tile([P, nch * 8], f32, tag="lr")
        nc.vector.tensor_scalar_mul(out=lr[:], in0=z[:], scalar1=0.2)
        nc.vector.tensor_tensor(out=lr[:], in0=lr[:], in1=z[:],
                                op=mybir.AluOpType.max)
        ex = ep.tile([P, nch * 8], bfl, tag="ex")
        nc.scalar.activation(out=ex[:], in_=lr[:],
                             func=mybir.ActivationFunctionType.Exp)
        S = ep.tile([P, nch * P], bfl, tag="S")
        nc.vector.tensor_tensor(
            out=S[:].rearrange("p (c j) -> p c j", j=P),
            in0=iota_t[:].unsqueeze(1).to_broadcast([P, nch, P]),
            in1=dl[:].unsqueeze(2).to_broadcast([P, nch, P]),
            op=mybir.AluOpType.is_equal)
        V = ep.tile([P, nch * FCOL], bfl, tag="V")
        exv = ex[:].rearrange("p (c h) -> p c h", h=8)
        nc.vector.tensor_tensor(
            out=V[:].rearrange("p (c e) -> p c e", e=FCOL)[:, :, :FEAT]
                 .rearrange("p c (h f) -> p c h f", f=FEAT // 8),
            in0=Gv[:, :, :FEAT].rearrange("p c (h f) -> p c h f", f=FEAT // 8),
            in1=exv.unsqueeze(3).to_broadcast([P, nch, 8, FEAT // 8]),
            op=mybir.AluOpType.mult)
        nc.vector.tensor_copy(
            out=V[:].rearrange("p (c e) -> p c e", e=FCOL)[:, :, FEAT:FCOL],
            in_=exv)
        ps = pp1.tile([P, FCOL], f32, tag="eps")
        for cch in range(nch):
            nc.tensor.matmul(
                ps[:], lhsT=S[:, cch * P:(cch + 1) * P],
                rhs=V[:, cch * FCOL:(cch + 1) * FCOL],
                start=(cch == 0), stop=(cch == nch - 1))
        ro = ep.tile([P, FCOL], f32, tag="ro")
        nc.vector.tensor_copy(out=ro[:], in_=ps[:])
        m = sc["nslots"]
        nc.sync.dma_start(out=t_raw[sc["start"]:sc["start"] + m, :],
                          in_=ro[:m, :])


# ----------------------------------------------------------------- driver

def _prepare(inputs):
    x, src, dst = inputs["x"], inputs["src"], inputs["dst"]
    n_nodes, IN_F = x.shape
    H, F1 = inputs["al1"].shape
    F2 = inputs["al2"].shape[1]
    pl = _plan(np.asarray(dst), n_nodes)
    sc1, CA1, CB1 = _streams_for_layer(pl, np.asarray(src), np.asarray(dst), 1)
    sc2, CA2, CB2 = _streams_for_layer(pl, np.asarray(src), np.asarray(dst), 2)
    dcore = {"sc1": sc1, "CA1": CA1, "CB1": CB1,
             "sc2": sc2, "CA2": CA2, "CB2": CB2,
             "l1": [], "l2": []}
    for c in range(NC_CORES):
        dcore["l1"].append(_pack_core_layer(pl, sc1[c], CA1, CB1))
        dcore["l2"].append(_pack_core_layer(pl, sc2[c], CA2, CB2))
    return pl, dcore, (n_nodes, IN_F, H, F1, F2)


def kernel(x, src, dst, W1, al1, ar1, b1, W2, al2, ar2, b2, _mode="hw"):
    inputs = dict(x=np.asarray(x), src=np.asarray(src), dst=np.asarray(dst),
                  W1=np.asarray(W1), al1=np.asarray(al1), ar1=np.asarray(ar1),
                  b1=np.asarray(b1), W2=np.asarray(W2), al2=np.asarray(al2),
                  ar2=np.asarray(ar2), b2=np.asarray(b2))
    pl, dcore, (n_nodes, IN_F, H, F1, F2) = _prepare(inputs)
    if _mode == "sim":
        return _simulate(pl, inputs, dcore)

    from concourse.bass_utils import run_bass_kernel_spmd
    rhs1, rhs2, corr2 = _host_weights(inputs["W1"], inputs["al1"], inputs["ar1"],
                                      inputs["W2"], inputs["al2"], inputs["ar2"],
                                      F1, H, F2)
    nc = _build_bass(pl, dcore, IN_F, H, F1, F2)
    iota = np.tile(np.arange(P, dtype=np.float32).astype(bf16)[None, :], (P, 1))
    corr2_rep = np.tile(corr2[None, :], (P, 1)).astype(np.float32)
    b1rep = np.tile(inputs["b1"].astype(np.float32)[None, :], (P, 1))
    in_maps = []
    for c in range(NC_CORES):
        iA1, iB1, iE1, dl1, cn1 = dcore["l1"][c]
        iA2, iB2, iE2, dl2, cn2 = dcore["l2"][c]
        in_maps.append({
            "xT": _build_xT(pl, c, inputs["x"], IN_F),
            "rhs1": rhs1, "rhs2": rhs2, "corr2": corr2_rep, "b1rep": b1rep,
            "iota": iota,
            "iA1": iA1, "iB1": iB1, "iE1": iE1, "dl1": dl1, "cn1": cn1,
            "iA2": iA2, "iB2": iB2, "iE2": iE2, "dl2": dl2, "cn2": cn2,
        })
    import os
    trace_kwargs = {}
    if os.environ.get("BASS_TRACE"):
        trace_kwargs = dict(trace=True, tmpdir=os.environ.get("BASS_TRACE_DIR") or None)
    res = run_bass_kernel_spmd(nc, in_maps, core_ids=list(range(NC_CORES)),
                               **trace_kwargs)
    global LAST_RESULTS
    LAST_RESULTS = res
    out = np.zeros((n_nodes, H, F2), np.float32)
    b2r = inputs["b2"].astype(np.float32).reshape(1, H, F2)
    for c in range(NC_CORES):
        raw = res.results[c]["raw2"]
        o2 = raw[:, :H * F2].reshape(pl.D, H, F2) / \
            np.maximum(raw[:, H * F2:], EPS)[:, :, None] + b2r
        pc = np.asarray(pl.percore[c])
        mask = pc >= 0
        out[pc[mask]] = o2[mask]
    return out

